# revision 33
# baseline (speedup 1.0000x reference)
"""MultiHeadAttention (qk-LayerNorm + RoPE) Trainium2 kernel, 8 NeuronCores.

Sharding: batch (4) x sequence-half (2), with K/V computed redundantly on
both cores of a batch pair. Core c handles batch c//2 and query/output rows
512*(c%2)..512*(c%2)+511 for ALL 16 heads, so the output rows are disjoint
across cores and NO collective is needed (on-device collectives cost
~20-60us each in trigger+transfer on this stack, and their run-to-run
variance dominated the previous head-sharded design's tail).

Rank-dependence lives entirely in host-sliced inputs (the q-row slab and
its rope tables); the on-device program is identical on every core.

Perf notes:
- all matmuls bf16 (float32r lowers to fp32_mode=HIGH on HW, ~4x slower)
- host prepacking: every input is a few large contiguous DMAs
- LN mean folded into Wq/Wk on the host (per-head column-mean subtracted),
  so on-chip LN is just sum-of-squares + rsqrt
- softmax denominator: ones-column rides the ctx matmul; the denom row is
  copied to SBUF, bounced through DRAM to broadcast across 64 partitions,
  then reciprocal_approx_fast (plain DVE reciprocal is ~8 cyc/elem,
  single-lane; custom DVE ops misread PSUM, hence the SBUF staging)
- software-pipelined emission keeps the PE queue from stalling behind
  ACT/DVE work (transposes lag one tile; ctx matmuls lag one step), which
  also keeps the HAM clock gate at 2.4 GHz; junk ident-transposes bridge
  the phase boundary, and a dummy exp preloads the ACT exp table there
- heads are processed in pairs sharing one [128,1024] PSUM scores tile so
  each exp call stays maximal (the ~352ns/op ACT overhead is the pacer)
"""
import sys

for _p in ("/opt/trn_rl_repo", "/root/.axon_site", "/root/.axon_site/_ro/trn_rl_repo",
           "/root/.axon_site/_ro/pypackages"):
    if _p not in sys.path:
        sys.path.append(_p)

import numpy as np
import ml_dtypes

import concourse.bass as bass
import concourse.tile as tile
from concourse import bacc, mybir
from concourse.bass_utils import run_bass_kernel_spmd
from concourse.masks import make_identity

F32 = mybir.dt.float32
BF16 = mybir.dt.bfloat16
P = 128
B, L, C, H, D = 4, 1024, 1024, 16, 64
NT = L // P     # 8 key/value token tiles
NQ = 4          # 4 query token tiles (own half)
NCK = C // P    # 8 contraction tiles
NPR = H // 2    # 8 head pairs
QL = NQ * P     # 512 own query rows
THETA = 50000.0
EPS = 1e-5
BF = ml_dtypes.bfloat16

_NC_CACHE = {}


def _build_nc():
    nc = bacc.Bacc("TRN2", target_bir_lowering=False, debug=False, num_devices=8)

    x_d = nc.dram_tensor("xp", [P, NT, NCK * P], BF16, kind="ExternalInput")
    xq_d = nc.dram_tensor("xqp", [P, NQ, NCK * P], BF16, kind="ExternalInput")
    wq_d = nc.dram_tensor("wqp", [P, NCK * C], BF16, kind="ExternalInput")
    wk_d = nc.dram_tensor("wkp", [P, NCK * C], BF16, kind="ExternalInput")
    wv_d = nc.dram_tensor("wvp", [P, NCK * C], BF16, kind="ExternalInput")
    wo_d = nc.dram_tensor("wop", [P, NPR * C], BF16, kind="ExternalInput")
    tbk_d = nc.dram_tensor("tbkp", [P, NT * 2 * D], BF16, kind="ExternalInput")
    tbq_d = nc.dram_tensor("tbqp", [P, NQ * 2 * D], BF16, kind="ExternalInput")
    out_d = nc.dram_tensor("out", [QL, C], F32, kind="ExternalOutput")

    with tile.TileContext(nc) as tc:
        with (
            tc.tile_pool(name="const", bufs=1) as constp,
            tc.tile_pool(name="w", bufs=1) as wpool,
            tc.tile_pool(name="big", bufs=1) as bigp,
            tc.tile_pool(name="sq", bufs=2) as sqp,
            tc.tile_pool(name="stat", bufs=2) as statp,
            tc.tile_pool(name="scr", bufs=2) as scrp,
            tc.tile_pool(name="rope", bufs=2) as ropep,
            tc.tile_pool(name="exp", bufs=4) as expp,
            tc.tile_pool(name="fin", bufs=2) as finp,
            tc.tile_pool(name="dram", bufs=2, space="DRAM") as dram,
        ):
            # ---- inputs, ordered so the first matmuls can start early ----
            wk_t = wpool.tile([P, NCK, C], BF16)
            nc.sync.dma_start(wk_t[:], wk_d.ap().rearrange("p (k o) -> p k o", k=NCK))
            xt = bigp.tile([P, NT, NCK, P], BF16)
            nc.sync.dma_start(
                xt[:, 0, :, :],
                x_d.ap().rearrange("p t (k u) -> p t k u", k=NCK)[:, 0, :, :])
            wv_t = wpool.tile([P, NCK, C], BF16)
            nc.sync.dma_start(wv_t[:], wv_d.ap().rearrange("p (k o) -> p k o", k=NCK))
            for ti in range(1, 4):
                nc.sync.dma_start(
                    xt[:, ti, :, :],
                    x_d.ap().rearrange("p t (k u) -> p t k u", k=NCK)[:, ti, :, :])
            # wq is not needed until the Q pass (~90us in); keep it behind the
            # early x tiles so tile 1..3's matmuls aren't stuck behind 2MB
            wq_t = wpool.tile([P, NCK, C], BF16)
            nc.sync.dma_start(wq_t[:], wq_d.ap().rearrange("p (k o) -> p k o", k=NCK))
            for ti in range(4, NT):
                nc.sync.dma_start(
                    xt[:, ti, :, :],
                    x_d.ap().rearrange("p t (k u) -> p t k u", k=NCK)[:, ti, :, :])
            xq = bigp.tile([P, NQ, NCK, P], BF16)
            nc.sync.dma_start(xq[:],
                              xq_d.ap().rearrange("p t (k u) -> p t k u", k=NCK))
            tbk_t = constp.tile([P, NT, 2, D], BF16)
            nc.sync.dma_start(tbk_t[:],
                              tbk_d.ap().rearrange("p (t i d) -> p t i d", t=NT, i=2))
            tbq_t = constp.tile([P, NQ, 2, D], BF16)
            nc.sync.dma_start(tbq_t[:],
                              tbq_d.ap().rearrange("p (t i d) -> p t i d", t=NQ, i=2))
            wo_t = wpool.tile([P, NPR, C], BF16)
            nc.sync.dma_start(wo_t[:], wo_d.ap().rearrange("p (r o) -> p r o", r=NPR))

            ident = constp.tile([P, P], BF16)
            make_identity(nc, ident)
            eps_t = constp.tile([P, 1], F32)
            nc.vector.memset(eps_t[:], EPS)

            # v with a ones column appended per head: [keys, j, head, 65]
            v_sb = bigp.tile([P, NT, H, D + 1], BF16)
            nc.gpsimd.memset(v_sb[:, :, :, D:D + 1], 1.0)

            kT_pack = bigp.tile([P, NPR, L], BF16)
            qT_pack = bigp.tile([P, NPR, QL], BF16)
            ctxT = bigp.tile([P, NPR, QL], BF16)

            # ------- Phase 1: K,V then Q projections + LN + RoPE ------------
            with tc.tile_pool(name="ps1", bufs=2, space="PSUM") as ps1, \
                 tc.tile_pool(name="pst", bufs=2, space="PSUM") as pstp:
                pend = []

                def emit_transposes(ti, rope_t, dstpack):
                    pst = pstp.tile([P, NPR, P], BF16, tag="pst")
                    for pr in range(NPR):
                        nc.tensor.transpose(
                            pst[:, pr, :],
                            rope_t[:, 2 * pr:2 * pr + 2, :].rearrange(
                                "p h d -> p (h d)"),
                            ident[:])
                    nc.scalar.copy(dstpack[:, :, bass.ts(ti, P)], pst[:])

                def emit_proj_tile(ti, src, w_, i_tb, tb, dstpack, with_v):
                    ps_ = ps1.tile([P, C], F32, tag="ps")
                    for ck in range(NCK):
                        for m in range(2):
                            nc.tensor.matmul(
                                ps_[:, bass.ts(m, 512)], src[:, ti, ck, :],
                                w_[:, ck, bass.ts(m, 512)],
                                start=(ck == 0), stop=(ck == NCK - 1))
                    if with_v:
                        # bufs=1 slot: the next tile's psv matmuls trail the
                        # v-copy by a full tile of PE work, so no stall
                        psv = ps1.tile([P, C], F32, tag="psv", bufs=1)
                        for ck in range(NCK):
                            for m in range(2):
                                nc.tensor.matmul(
                                    psv[:, bass.ts(m, 512)], src[:, ti, ck, :],
                                    wv_t[:, ck, bass.ts(m, 512)],
                                    start=(ck == 0), stop=(ck == NCK - 1))
                    if pend:
                        emit_transposes(*pend.pop())
                    if with_v:
                        nc.scalar.copy(
                            v_sb[:, ti, :, 0:D],
                            psv[:].rearrange("p (h d) -> p h d", d=D))

                    # LN: host-centered weights => mean 0; sum-of-squares only
                    sq = sqp.tile([P, C], F32, tag="sq")
                    nc.scalar.square(sq[:], ps_[:])
                    ssq = statp.tile([P, H], F32, tag="ssq")
                    nc.vector.reduce_sum(
                        ssq[:], sq[:].rearrange("p (h d) -> p h d", d=D),
                        axis=mybir.AxisListType.X)
                    std = statp.tile([P, H], F32, tag="std")
                    nc.scalar.activation(std[:], ssq[:],
                                         mybir.ActivationFunctionType.Sqrt,
                                         bias=eps_t[:], scale=1.0 / D)
                    inv = statp.tile([P, H], F32, tag="inv")
                    nc.vector.reciprocal(inv[:], std[:])

                    inv_b = inv[:].rearrange("p h -> p h ()").to_broadcast((P, H, D))
                    a_b = tb[:, ti, 0, :].rearrange("p d -> p () d").to_broadcast(
                        (P, H, D))
                    y = scrp.tile([P, H, D], BF16, tag="y")
                    nc.vector.tensor_mul(
                        y[:], ps_[:].rearrange("p (h d) -> p h d", d=D), inv_b)
                    t1 = scrp.tile([P, H, D], BF16, tag="t1")
                    nc.vector.tensor_mul(t1[:], y[:], a_b)
                    r2 = scrp.tile([P, H, D], BF16, tag="r2")
                    h_ = D // 2
                    nc.vector.tensor_mul(
                        r2[:, :, 0:h_], y[:, :, h_:D],
                        tb[:, ti, 1, 0:h_].rearrange("p d -> p () d").to_broadcast(
                            (P, H, h_)))
                    nc.vector.tensor_mul(
                        r2[:, :, h_:D], y[:, :, 0:h_],
                        tb[:, ti, 1, h_:D].rearrange("p d -> p () d").to_broadcast(
                            (P, H, h_)))
                    rope_t = ropep.tile([P, H, D], BF16, tag="rope")
                    nc.vector.tensor_add(rope_t[:], t1[:], r2[:])
                    pend.append((ti, rope_t, dstpack))

                for ti in range(NT):
                    emit_proj_tile(ti, xt, wk_t, 1, tbk_t, kT_pack, True)
                for ti in range(NQ):
                    emit_proj_tile(ti, xq, wq_t, 0, tbq_t, qT_pack, False)
                emit_transposes(*pend.pop())
                # junk transposes bridge the phase boundary (HAM stays warm)
                for _ in range(12):
                    junk = pstp.tile([P, NPR, P], BF16, tag="pst")
                    for pr in range(NPR):
                        nc.tensor.transpose(junk[:, pr, :], ident[:], ident[:])
                # preload the exp ACT table set during the same bubble
                escr = statp.tile([P, 1], F32, tag="escr")
                nc.scalar.activation(escr[:], eps_t[:],
                                     mybir.ActivationFunctionType.Exp)

            # ------- Phase 2: attention, head pairs, pipelined --------------
            with tc.tile_pool(name="pss", bufs=2, space="PSUM") as pssp, \
                 tc.tile_pool(name="psc", bufs=2, space="PSUM") as pscp:
                psc_of, expT_of = {}, {}

                def emit_sc(pr, j):
                    pss = pssp.tile([P, 2, QL], F32, tag="pss")
                    for sub in range(2):
                        lo = D * sub
                        nc.tensor.matmul(
                            pss[:, sub, :],
                            kT_pack[lo:lo + D, pr, bass.ts(j, P)],
                            qT_pack[lo:lo + D, pr, :],
                            start=True, stop=True)
                    expT = expp.tile([P, 2, QL], BF16, tag="expT")
                    nc.scalar.activation(expT[:], pss[:],
                                         mybir.ActivationFunctionType.Exp,
                                         scale=float(D) ** -0.5)
                    expT_of[(pr, j)] = expT

                def emit_ctx(pr, j):
                    expT = expT_of.pop((pr, j))
                    for sub in range(2):
                        nc.tensor.matmul(
                            psc_of[(pr, sub)][:],
                            v_sb[:, j, 2 * pr + sub, :],
                            expT[:, sub, :],
                            start=(j == 0), stop=(j == NT - 1))

                def emit_drain(pr, sub):
                    psc = psc_of.pop((pr, sub))
                    den_row = finp.tile([D + 1, QL], F32, tag="denrow")
                    nc.vector.tensor_copy(den_row[D:D + 1, :], psc[D:D + 1, :])
                    dden = dram.tile([1, QL], F32, tag="dden",
                                     name=f"dden{pr}_{sub}")
                    nc.sync.dma_start(dden[:], den_row[D:D + 1, :])
                    den_b = finp.tile([D, QL], F32, tag="denb")
                    nc.sync.dma_start(den_b[:], dden[0:1, :].to_broadcast((D, QL)))
                    rb = finp.tile([D, QL], F32, tag="rb")
                    nc.vector.reciprocal_approx_fast(rb[:], den_b[:])
                    if sub == 0:
                        nc.vector.tensor_mul(ctxT[0:D, pr, :], psc[0:D, :], rb[:])
                    else:
                        stage = finp.tile([D, QL], BF16, tag="stage")
                        nc.vector.tensor_mul(stage[:], psc[0:D, :], rb[:])
                        nc.sync.dma_start(ctxT[D:2 * D, pr, :], stage[:])

                steps = [(pr, j) for pr in range(NPR) for j in range(NT)]
                for idx, (pr, j) in enumerate(steps):
                    if j == 0:
                        for sub in range(2):
                            psc_of[(pr, sub)] = pscp.tile(
                                [D + 1, QL], F32, tag=f"psc{sub}",
                                name=f"psc{pr}_{sub}")
                    emit_sc(pr, j)
                    if idx >= 1:
                        ppr, pj = steps[idx - 1]
                        emit_ctx(ppr, pj)
                        if pj == NT - 1:
                            emit_drain(ppr, 0)
                            emit_drain(ppr, 1)
                emit_ctx(NPR - 1, NT - 1)
                emit_drain(NPR - 1, 0)
                emit_drain(NPR - 1, 1)

            # ------- Phase 3: own-half output projection --------------------
            # pairs 0..6 accumulate first (their ctx is drained ~9us before
            # the last pair's), so the PE overlaps the final drain chain;
            # pair 7's contribution closes each tile.
            with tc.tile_pool(name="pso", bufs=1, space="PSUM") as psop:
                psos = []
                for tl in range(NQ):
                    pso = psop.tile([P, C], F32, tag=f"pso{tl}", name=f"pso{tl}")
                    psos.append(pso)
                    for m in range(2):
                        for pr in range(NPR - 1):
                            nc.tensor.matmul(
                                pso[:, bass.ts(m, 512)],
                                ctxT[:, pr, bass.ts(tl, P)],
                                wo_t[:, pr, bass.ts(m, 512)],
                                start=(pr == 0), stop=False)
                for tl in range(NQ):
                    pso = psos[tl]
                    for m in range(2):
                        nc.tensor.matmul(
                            pso[:, bass.ts(m, 512)],
                            ctxT[:, NPR - 1, bass.ts(tl, P)],
                            wo_t[:, NPR - 1, bass.ts(m, 512)],
                            start=False, stop=True)
                    out_sb = finp.tile([P, C], F32, tag="out")
                    if tl % 2 == 0:
                        nc.scalar.copy(out_sb[:], pso[:])
                    else:
                        nc.vector.tensor_copy(out_sb[:], pso[:])
                    nc.sync.dma_start(out_d.ap()[bass.ts(tl, P), :], out_sb[:])

    nc.compile()
    return nc


def _rope_tables(w, b):
    """A[t,d], B[t,d] with the rotate-half sign folded into B."""
    inv_freq = 1.0 / THETA ** (np.arange(0, D, 2, dtype=np.float64) / D)
    freqs = np.arange(L, dtype=np.float64)[:, None] * inv_freq[None, :]
    freqs = np.concatenate([freqs, freqs], axis=1)           # [L, D]
    cos, sin = np.cos(freqs), np.sin(freqs)
    w = w.astype(np.float64)
    w_rot = np.concatenate([w[D // 2:], w[:D // 2]])
    sgn = np.concatenate([-np.ones(D // 2), np.ones(D // 2)])
    A = (cos * w[None, :]).astype(np.float32)
    Bt = (sin * w_rot[None, :] * sgn[None, :]).astype(np.float32)
    if np.any(b != 0):
        raise NotImplementedError("nonzero qk-norm bias not supported")
    return A, Bt


def _center_heads(W):
    """Subtract the per-head column mean so projected q/k are zero-mean."""
    Wh = W.reshape(H, D, C)
    return (Wh - Wh.mean(axis=1, keepdims=True)).reshape(C, C)


def _pack_x(xb):
    """[rows, C] -> [p, tiles, ck*128] with x[p, t, ck, u] = xb[t*128+u, ck*128+p]."""
    nt = xb.shape[0] // P
    a = xb.reshape(nt, P, NCK, P)
    return np.ascontiguousarray(a.transpose(3, 0, 2, 1)).reshape(P, nt, -1).astype(BF)


def _pack_w(Wm):
    """[C_out, C] -> [p, ck*C_out] with w[p, ck, o] = Wm[o, ck*128+p]."""
    wT = Wm.T.reshape(NCK, P, C)
    return np.ascontiguousarray(wT.transpose(1, 0, 2)).reshape(P, -1).astype(BF)


def _pack_tb(A, Bt):
    """[rows, D] x2 -> [p, t*2*D]."""
    nt = A.shape[0] // P
    tb = np.stack([A, Bt], axis=1).reshape(nt, P, 2, D)
    return np.ascontiguousarray(tb.transpose(1, 0, 2, 3)).reshape(P, -1).astype(BF)


def kernel(**inputs):
    x = np.asarray(inputs["q"], dtype=np.float32)
    Wq = _center_heads(np.asarray(inputs["Wq"], dtype=np.float32))
    Wk = _center_heads(np.asarray(inputs["Wk"], dtype=np.float32))
    Wv = np.asarray(inputs["Wv"], dtype=np.float32)
    Wo = np.asarray(inputs["Wo"], dtype=np.float32)
    bo = np.asarray(inputs["bo"], dtype=np.float32)
    assert not np.any(bo != 0), "nonzero output bias not supported"

    Aq, Bq = _rope_tables(np.asarray(inputs["qn_w"], np.float32),
                          np.asarray(inputs["qn_b"], np.float32))
    Ak, Bk = _rope_tables(np.asarray(inputs["kn_w"], np.float32),
                          np.asarray(inputs["kn_b"], np.float32))

    wqp, wkp, wvp = _pack_w(Wq), _pack_w(Wk), _pack_w(Wv)
    # wo[r, pr, o] = Wo.T[pr*128 + r, o]  (ctx head pairs stacked on 128 rows)
    wo8 = Wo.T.reshape(NPR, 2 * D, C)
    wop = np.ascontiguousarray(wo8.transpose(1, 0, 2)).reshape(2 * D, -1).astype(BF)
    tbk = _pack_tb(Ak, Bk)

    if "nc" not in _NC_CACHE:
        _NC_CACHE["nc"] = _build_nc()
    nc = _NC_CACHE["nc"]

    in_maps = []
    for c in range(8):
        b_, r = c // 2, c % 2
        rows = slice(QL * r, QL * (r + 1))
        in_maps.append({
            "xp": _pack_x(x[b_]),
            "xqp": _pack_x(x[b_][rows]),
            "wqp": wqp, "wkp": wkp, "wvp": wvp, "wop": wop,
            "tbkp": tbk,
            "tbqp": _pack_tb(Aq[rows], Bq[rows]),
        })

    res = run_bass_kernel_spmd(nc, in_maps, core_ids=list(range(8)))
    out = np.empty((B, L, C), dtype=np.float32)
    for b_ in range(B):
        out[b_, 0:QL] = res.results[2 * b_]["out"]
        out[b_, QL:L] = res.results[2 * b_ + 1]["out"]
    return out
